# revision 7
# baseline (speedup 1.0000x reference)
"""Autoformer attention block kernel for 8 TRN2 NeuronCores.

Math reduction (validated vs reference to 2e-7):
 - output = x + AutoCorrelation(series_decomp(LN(x)))  (final decomp s2+t2 == x2)
 - mean over lags of the FFT cross-correlation == (sum_t Q)*(sum_t K)  (DC bin),
   so no FFT is needed: top-k stats come from column sums of `seasonal`.
 - beta cancels exactly (band operator has row-sum 1); gamma folds into
   Wvo = diag(gamma) @ Wv @ Wo and the qsum scaling.
 - delay aggregation = 64-tap circular FIR along time with data-dependent
   weights -> banded Toeplitz matmul on the TensorEngine.

Sharding: data-parallel over batch (B=8 -> 8 cores); one [64]-float AllReduce
for the global top-40 channel selection.
"""

import sys

if "/opt/trn_rl_repo" not in sys.path:
    sys.path.insert(0, "/opt/trn_rl_repo")

import numpy as np

L = 3072
D = 512
NT = L // 128  # 24 time tiles
H = 8
DK = 64
KTOP = 40
PAD = 12  # (25-1)//2
EPS = 1e-5
NCORES = 8
HL = float(H * L)

_CACHE = {}


def _np_consts():
    t = np.arange(L)
    lo = np.maximum(t - PAD, 0)
    hi = np.minimum(t + PAD + 1, L)
    inv = 1.0 / (hi - lo).astype(np.float64)

    # phi[s] = 1 - sum over t in the window around s of 1/win(t); nonzero only
    # in the first/last 24 positions.
    phi = np.ones(L, np.float64)
    for s in range(L):
        a = max(0, s - PAD)
        b = min(L, s + PAD + 1)
        phi[s] -= inv[a:b].sum()

    # band lhsT consts, all [128,128], K = full z tile, zero-padded:
    # chunk X in {A: s = t0-128+j, B: s = t0+j, C: s = t0+128+j}:
    #   M[j, p] = delta(s, t0+p) - [|t0+p - s| <= PAD] / win(t0+p)
    def band(t0, soff):
        j = np.arange(128)[:, None]
        p = np.arange(128)[None, :]
        s = soff + j
        tp = t0 + p
        m = (np.abs(tp - s) <= PAD) & (s >= 0) & (s + soff - soff < L)
        M = -(m * inv[np.clip(tp, 0, L - 1)])
        M = M + (s == tp) * 1.0
        return np.ascontiguousarray(M, np.float32)

    t0m = 1280  # any interior tile
    b_A = band(t0m, t0m - 128)
    b_C = band(t0m, t0m + 128)
    b_Bf = band(0, 0)
    b_Bm = band(t0m, t0m)
    b_Bl = band(L - 128, L - 128)
    phi_h = np.zeros((128, 1), np.float32)
    phi_h[:24, 0] = phi[:24]
    phi_t = np.zeros((128, 1), np.float32)
    phi_t[104:, 0] = phi[-24:]
    ident = np.eye(128, dtype=np.float32)
    return b_A, b_C, b_Bf, b_Bm, b_Bl, phi_h, phi_t, ident


def _build():
    import concourse.bass as bass
    import concourse.tile as tile
    import concourse.mybir as mybir
    from concourse import bacc
    import bass_rust
    import ml_dtypes

    dt = mybir.dt
    f32 = dt.float32
    f32r = dt.float32r
    bf16 = dt.bfloat16
    AF = mybir.ActivationFunctionType
    ALU = mybir.AluOpType
    AX = mybir.AxisListType
    ts = bass.ts

    nc = bacc.Bacc(None, target_bir_lowering=False)

    xe = nc.dram_tensor("xb", [L, D], f32, kind="ExternalInput")
    wqe = nc.dram_tensor("Wq", [D, D], f32, kind="ExternalInput")
    wke = nc.dram_tensor("Wk", [D, D], f32, kind="ExternalInput")
    wve = nc.dram_tensor("Wv", [D, D], f32, kind="ExternalInput")
    woe = nc.dram_tensor("Wo", [D, D], f32, kind="ExternalInput")
    bqe = nc.dram_tensor("bq", [D], f32, kind="ExternalInput")
    bke = nc.dram_tensor("bk", [D], f32, kind="ExternalInput")
    bve = nc.dram_tensor("bv", [D], f32, kind="ExternalInput")
    boe = nc.dram_tensor("bo", [D], f32, kind="ExternalInput")
    gme = nc.dram_tensor("gamma", [D], f32, kind="ExternalInput")
    oute = nc.dram_tensor("out", [L, D], f32, kind="ExternalOutput")

    bA, bC, bBf, bBm, bBl, phi_h, phi_t, ident = _np_consts()
    bf = ml_dtypes.bfloat16
    cbA = nc.inline_tensor(bA.astype(bf), "c_bA")
    cbC = nc.inline_tensor(bC.astype(bf), "c_bC")
    cbBf = nc.inline_tensor(bBf.astype(bf), "c_bBf")
    cbBm = nc.inline_tensor(bBm.astype(bf), "c_bBm")
    cbBl = nc.inline_tensor(bBl.astype(bf), "c_bBl")
    cphih = nc.inline_tensor(phi_h.astype(bf), "c_phih")
    cphit = nc.inline_tensor(phi_t.astype(bf), "c_phit")
    cid = nc.inline_tensor(ident.astype(bf), "c_id")
    cones1x64 = nc.inline_tensor(np.ones((1, 64), np.float32), "c_o64")
    cones1x128b = nc.inline_tensor(
        np.ones((1, 128), ml_dtypes.bfloat16), "c_o128b"
    )

    from contextlib import ExitStack

    with tile.TileContext(nc) as tc, ExitStack() as ctx:
        pc = ctx.enter_context(tc.tile_pool(name="consts", bufs=1))
        px = ctx.enter_context(tc.tile_pool(name="xarr", bufs=NT))
        pz = ctx.enter_context(tc.tile_pool(name="zroll", bufs=5))
        pz23 = ctx.enter_context(tc.tile_pool(name="z23", bufs=1))
        pvo = ctx.enter_context(tc.tile_pool(name="voarr", bufs=NT))
        pwvo = ctx.enter_context(tc.tile_pool(name="wvo", bufs=4))
        pwt = ctx.enter_context(tc.tile_pool(name="wtmp", bufs=4))
        pwork = ctx.enter_context(tc.tile_pool(name="work", bufs=3))
        psq = ctx.enter_context(tc.tile_pool(name="sqscr", bufs=2))
        pstt = ctx.enter_context(tc.tile_pool(name="stats", bufs=6))
        psm = ctx.enter_context(tc.tile_pool(name="smalls", bufs=2))
        pout = ctx.enter_context(tc.tile_pool(name="osb", bufs=3))
        pseasT = ctx.enter_context(tc.tile_pool(name="seasT", bufs=8))
        pdram = ctx.enter_context(tc.tile_pool(name="dram", bufs=1, space="DRAM"))
        qst = ctx.enter_context(tc.tile_pool(name="ps_st", bufs=1, space="PSUM"))
        qtp = ctx.enter_context(tc.tile_pool(name="ps_tp", bufs=1, space="PSUM"))
        qvo = ctx.enter_context(tc.tile_pool(name="ps_vo", bufs=2, space="PSUM"))
        qsm = ctx.enter_context(tc.tile_pool(name="ps_sm", bufs=2, space="PSUM"))
        qtap = ctx.enter_context(tc.tile_pool(name="ps_tap", bufs=2, space="PSUM"))

        # ---------------- constants to SBUF ----------------
        def cload(name, shape, src, dtype=f32):
            t = pc.tile(list(shape), dtype, tag=name)
            nc.sync.dma_start(t[:], src)
            return t

        bndA = cload("bndA", (128, 128), cbA[:, :], bf16)
        bndC = cload("bndC", (128, 128), cbC[:, :], bf16)
        bndBf = cload("bndBf", (128, 128), cbBf[:, :], bf16)
        bndBm = cload("bndBm", (128, 128), cbBm[:, :], bf16)
        bndBl = cload("bndBl", (128, 128), cbBl[:, :], bf16)
        phish = cload("phish", (128, 1), cphih[:, :], bf16)
        phist = cload("phist", (128, 1), cphit[:, :], bf16)
        idt = cload("idt", (128, 128), cid[:, :], bf16)
        o1x64 = cload("o1x64", (1, 64), cones1x64[:, :])
        o1x128b = cload("o1x128b", (1, 128), cones1x128b[:, :], bf16)

        gammaP = pc.tile([128, 4], f32, tag="gammaP")
        nc.sync.dma_start(gammaP[:], gme[:].rearrange("(a b) -> b a", b=128))
        bvP = pc.tile([128, 4], f32, tag="bvP")
        nc.sync.dma_start(bvP[:], bve[:].rearrange("(a b) -> b a", b=128))
        bqv = pc.tile([1, 512], f32, tag="bqv")
        nc.sync.dma_start(bqv[:], bqe[:])
        bkv = pc.tile([1, 512], f32, tag="bkv")
        nc.sync.dma_start(bkv[:], bke[:])
        bov = pc.tile([1, 512], f32, tag="bov")
        nc.sync.dma_start(bov[:], boe[:])
        bq_sc = pc.tile([1, 512], f32, tag="bq_sc")
        nc.scalar.mul(bq_sc[:], bqv[:], float(L))
        bk_sc = pc.tile([1, 512], f32, tag="bk_sc")
        nc.scalar.mul(bk_sc[:], bkv[:], float(L))

        ones64 = nc.const_aps.tensor(1.0, (64, 1))

        # toep scratch in DRAM; zero it early
        toep = pdram.tile([191, 128], bf16, tag="toep")
        zline = pc.tile([128, 128], bf16, tag="zline")
        nc.vector.memset(zline[:], 0.0)
        nc.sync.dma_start(toep[0:128, :], zline[:])
        nc.sync.dma_start(toep[128:191, :], zline[0:63, :])

        # ---------------- weight prep: Wvo = diag(gamma) Wv Wo, cvec ----------------
        wo_sb = []
        wv_sc = []
        for a in range(4):
            w = pwt.tile([128, 512], f32, tag="wo")
            nc.sync.dma_start(w[:], woe[ts(a, 128), :])
            wo_sb.append(w)
        wob = []
        for a in range(4):
            w = pwt.tile([128, 512], bf16, tag="wob")
            nc.scalar.copy(w[:], wo_sb[a][:])
            wob.append(w)
        for a in range(4):
            w = pwt.tile([128, 512], f32, tag="wv")
            nc.sync.dma_start(w[:], wve[ts(a, 128), :])
            ws = pwt.tile([128, 512], bf16, tag="wvs")
            nc.scalar.activation(
                ws[:], w[:], AF.Identity, scale=gammaP[:, a : a + 1]
            )
            wv_sc.append(ws)
        wvT = []
        for c in range(4):
            w = pwt.tile([128, 512], bf16, tag="wvT")
            wvT.append(w)
        for a in range(4):
            for c in range(4):
                tp = qtp.tile([128, 128], bf16)
                nc.tensor.transpose(tp[:], wv_sc[a][:, ts(c, 128)], idt[:])
                nc.vector.tensor_copy(wvT[c][:, ts(a, 128)], tp[:])
        wvo = []
        for a in range(4):
            vps = qvo.tile([128, 512], f32)
            for c in range(4):
                nc.tensor.matmul(
                    vps[:],
                    wvT[c][:, ts(a, 128)],
                    wob[c][:],
                    start=(c == 0),
                    stop=(c == 3),
                )
            w = pwvo.tile([128, 512], bf16, tag="wvo")
            nc.scalar.copy(w[:], vps[:])
            wvo.append(w)

        # cvec = bv @ Wo + bo (as bf16 row for the tap matmul)
        cps = qsm.tile([1, 512], f32, tag="sm")
        for c in range(4):
            nc.tensor.matmul(
                cps[:],
                bvP[:, c : c + 1],
                wo_sb[c][:],
                start=(c == 0),
                stop=(c == 3),
            )
        cv_sb = psm.tile([1, 512], f32, tag="cv")
        nc.vector.tensor_tensor(cv_sb[:], cps[:], bov[:], op=ALU.add)
        cvb = psm.tile([1, 512], bf16, tag="cvb")
        nc.vector.tensor_copy(cvb[:], cv_sb[:])

        # Wq/Wk for the tiny projections
        wq_sb = []
        wk_sb = []
        for a in range(4):
            w = pwt.tile([128, 512], f32, tag="wq")
            nc.sync.dma_start(w[:], wqe[ts(a, 128), :])
            wq_sb.append(w)
            w = pwt.tile([128, 512], f32, tag="wk")
            nc.sync.dma_start(w[:], wke[ts(a, 128), :])
            wk_sb.append(w)

        # ---------------- per-tile LN ----------------
        xt = [None] * NT
        zt = [None] * NT

        def emit_z(i):
            x = px.tile([128, 512], f32, tag="x")
            nc.sync.dma_start(x[:], xe[ts(i, 128), :])
            xt[i] = x
            st = pstt.tile([128, 8], f32, tag="st")
            nc.vector.tensor_reduce(
                st[:, 0:1], x[:], axis=AX.X, op=ALU.add
            )  # sum x
            sq = psq.tile([128, 512], f32, tag="sq")
            nc.scalar.activation(
                sq[:], x[:], AF.Square, accum_out=st[:, 1:2]
            )  # sum x^2
            nc.vector.tensor_scalar(
                st[:, 2:3], st[:, 0:1], 1.0 / D, None, op0=ALU.mult
            )  # mu
            nc.vector.tensor_scalar(
                st[:, 3:4], st[:, 2:3], st[:, 2:3], None, op0=ALU.mult
            )  # mu^2
            nc.vector.tensor_scalar(
                st[:, 4:5], st[:, 3:4], -1.0, EPS, op0=ALU.mult, op1=ALU.add
            )  # eps - mu^2
            nc.scalar.activation(
                st[:, 5:6], st[:, 1:2], AF.Sqrt, bias=st[:, 4:5], scale=1.0 / D
            )  # sd = sqrt(sxx/D + eps - mu^2)
            nc.vector.reciprocal(st[:, 6:7], st[:, 5:6])  # r
            nc.vector.tensor_scalar(
                st[:, 7:8], st[:, 2:3], st[:, 6:7], -1.0, op0=ALU.mult, op1=ALU.mult
            )  # -mu*r
            if i == 23:
                z = pz23.tile([128, 512], bf16, tag="z23")
            else:
                z = pz.tile([128, 512], bf16, tag="z")
            nc.scalar.activation(
                z[:], x[:], AF.Identity, bias=st[:, 7:8], scale=st[:, 6:7]
            )
            zt[i] = z

        # ---------------- per-tile seasonal + vo ----------------
        vo = [None] * NT

        def emit_seasonal(i):
            sps = qst.tile([128, 512], f32)
            if i == 0:
                chunks = [
                    (bndBf[:], zt[0][:, :]),
                    (bndC[:], zt[1][:, :]),
                ]
            elif i == NT - 1:
                chunks = [
                    (bndA[:], zt[22][:, :]),
                    (bndBl[:], zt[23][:, :]),
                ]
            else:
                chunks = [
                    (bndA[:], zt[i - 1][:, :]),
                    (bndBm[:], zt[i][:, :]),
                    (bndC[:], zt[i + 1][:, :]),
                ]
            nck = len(chunks)
            for k, (lt, rz) in enumerate(chunks):
                nc.tensor.matmul(
                    sps[:],
                    lt,
                    rz,
                    start=(k == 0),
                    stop=(k == nck - 1),
                )
            seas = pwork.tile([128, 512], bf16, tag="seas")
            nc.scalar.copy(seas[:], sps[:])
            sTl = []
            for c in range(4):
                tp = qtp.tile([128, 128], bf16)
                nc.tensor.transpose(tp[:], seas[:, ts(c, 128)], idt[:])
                sT = pseasT.tile([128, 128], bf16, tag="sT")
                nc.vector.tensor_copy(sT[:], tp[:])
                sTl.append(sT)
            vps = qvo.tile([128, 512], f32)
            for c in range(4):
                nc.tensor.matmul(
                    vps[:],
                    sTl[c][:],
                    wvo[c][:],
                    start=(c == 0),
                    stop=(c == 3),
                )
            v = pvo.tile([128, 512], bf16, tag="vo")
            nc.scalar.copy(v[:], vps[:])
            vo[i] = v

        # ---------------- head/tail z + qsum partials ----------------
        emit_z(0)
        qps = qsm.tile([128, 8], f32, tag="sm")
        for c in range(4):
            nc.tensor.matmul(
                qps[:, c : c + 1],
                zt[0][:, ts(c, 128)],
                phish[:],
                start=True,
                stop=True,
            )
        emit_z(23)
        for c in range(4):
            nc.tensor.matmul(
                qps[:, 4 + c : 5 + c],
                zt[23][:, ts(c, 128)],
                phist[:],
                start=True,
                stop=True,
            )

        # ---------------- mv -> collective -> weights -> toeplitz ----------------
        qs_t = psm.tile([128, 4], f32, tag="qst")
        nc.scalar.copy(qs_t[:], qps[:, 4:8])
        qs_g = psm.tile([128, 4], f32, tag="qsg")
        nc.vector.tensor_tensor(qs_g[:], qps[:, 0:4], qs_t[:], op=ALU.add)
        nc.vector.tensor_tensor(qs_g[:], qs_g[:], gammaP[:], op=ALU.mult)

        qs_ps = qsm.tile([1, 512], f32, tag="sm")
        for c in range(4):
            nc.tensor.matmul(
                qs_ps[:],
                qs_g[:, c : c + 1],
                wq_sb[c][:],
                start=(c == 0),
                stop=(c == 3),
            )
        qsv = psm.tile([1, 512], f32, tag="qsv")
        nc.vector.tensor_tensor(qsv[:], qs_ps[:], bq_sc[:], op=ALU.add)
        ks_ps = qsm.tile([1, 512], f32, tag="sm")
        for c in range(4):
            nc.tensor.matmul(
                ks_ps[:],
                qs_g[:, c : c + 1],
                wk_sb[c][:],
                start=(c == 0),
                stop=(c == 3),
            )
        ksv = psm.tile([1, 512], f32, tag="ksv")
        nc.vector.tensor_tensor(ksv[:], ks_ps[:], bk_sc[:], op=ALU.add)

        pr = psm.tile([1, 512], f32, tag="pr")
        nc.vector.tensor_tensor(pr[:], qsv[:], ksv[:], op=ALU.mult)
        mvr = psm.tile([1, 64], f32, tag="mvr")
        nc.vector.tensor_reduce(
            mvr[:], pr[:].rearrange("p (h c) -> p c h", h=H), axis=AX.X, op=ALU.add
        )
        mv = psm.tile([1, 64], f32, tag="mv")
        nc.scalar.mul(mv[:], mvr[:], 1.0 / HL)

        ccin = pdram.tile([64], f32, tag="ccin")
        ccout = pdram.tile([64], f32, tag="ccout")
        mvd = pdram.tile([64], f32, tag="mvd")
        nc.gpsimd.dma_start(ccin[:], mv[:])
        nc.gpsimd.collective_compute(
            "AllReduce",
            ALU.add,
            replica_groups=[list(range(NCORES))],
            ins=[ccin[:].opt()],
            outs=[ccout[:].opt()],
        )
        nc.gpsimd.dma_start(mvd[:], mv[:])
        g_row = psm.tile([1, 64], f32, tag="grow")
        nc.sync.dma_start(g_row[:], ccout[:])
        gP = psm.tile([64, 1], f32, tag="gP")
        nc.sync.dma_start(gP[:], ccout[:])
        mvP = psm.tile([64, 1], f32, tag="mvP")
        nc.sync.dma_start(mvP[:], mvd[:])

        gf_ps = qsm.tile([64, 64], f32, tag="sm")
        nc.tensor.matmul(gf_ps[:], o1x64[:], g_row[:], start=True, stop=True)
        sc = psm.tile([64, 8], f32, tag="scm")
        cmp = psm.tile([64, 64], f32, tag="cmp")
        nc.vector.tensor_tensor(
            cmp[:], gf_ps[:], gP[:].to_broadcast((64, 64)), op=ALU.is_gt
        )
        nc.vector.tensor_reduce(sc[:, 0:1], cmp[:], axis=AX.X, op=ALU.add)  # rank
        nc.vector.tensor_scalar(
            sc[:, 1:2], sc[:, 0:1], KTOP - 0.5, None, op0=ALU.is_lt
        )  # mask
        nc.scalar.activation(sc[:, 2:3], mvP[:], AF.Exp)
        nc.vector.tensor_tensor(sc[:, 3:4], sc[:, 2:3], sc[:, 1:2], op=ALU.mult)
        s_ps = qsm.tile([1, 1], f32, tag="sm")
        nc.tensor.matmul(s_ps[:], sc[:, 3:4], ones64, start=True, stop=True)
        rs = psm.tile([1, 1], f32, tag="rs")
        nc.vector.reciprocal(rs[:], s_ps[:])
        rsf_ps = qsm.tile([64, 1], f32, tag="sm")
        nc.tensor.matmul(rsf_ps[:], o1x64[:], rs[:], start=True, stop=True)
        wf = psm.tile([64, 1], f32, tag="wf")
        nc.vector.tensor_tensor(wf[:], sc[:, 3:4], rsf_ps[:], op=ALU.mult)
        wrep = psm.tile([64, 128], bf16, tag="wrep")
        nc.vector.tensor_copy(wrep[:], wf[:].to_broadcast((64, 128)))

        # scatter wrep into the toeplitz band: toep[j, p] = wf[j - p]
        sc_ap = toep[:].flatten()
        sc_ap.ap = bass_rust.VecI64Pair([[128, 64], [129, 128]])
        nc.sync.dma_start(sc_ap, wrep[:])
        toepA = pc.tile([128, 128], bf16, tag="toepA")
        nc.sync.dma_start(toepA[:], toep[0:128, :])
        toepB = pc.tile([63, 128], bf16, tag="toepB")
        nc.sync.dma_start(toepB[:], toep[128:191, :])

        # ---------------- tap + residual + output ----------------
        def emit_tap(i):
            tps = qtap.tile([128, 512], f32)
            nc.tensor.matmul(tps[:], toepA[:], vo[i][:], start=True, stop=False)
            nc.tensor.matmul(
                tps[:],
                toepB[:],
                vo[(i + 1) % NT][0:63, :],
                start=False,
                stop=False,
            )
            nc.tensor.matmul(tps[:], o1x128b[:], cvb[:], start=False, stop=True)
            osb = pout.tile([128, 512], f32, tag="osb")
            nc.vector.tensor_tensor(osb[:], xt[i][:], tps[:], op=ALU.add)
            nc.sync.dma_start(oute[ts(i, 128), :], osb[:])

        # ---------------- main pipeline ----------------
        emit_z(1)
        emit_seasonal(0)
        for i in range(1, 22):
            emit_z(i + 1)
            emit_seasonal(i)
            emit_tap(i - 1)
        emit_seasonal(22)
        emit_tap(21)
        emit_seasonal(23)
        emit_tap(22)
        emit_tap(23)

    nc.finalize()
    return nc


def _get_nc():
    if "nc" not in _CACHE:
        _CACHE["nc"] = _build()
    return _CACHE["nc"]


def kernel_ext(inputs, trace=False):
    from concourse.bass_utils import run_bass_kernel_spmd

    nc = _get_nc()
    x = np.ascontiguousarray(inputs["x"], np.float32)
    common = {
        k: np.ascontiguousarray(inputs[k], np.float32)
        for k in ["Wq", "Wk", "Wv", "Wo", "bq", "bk", "bv", "bo", "gamma"]
    }
    in_maps = [{"xb": x[i], **common} for i in range(NCORES)]
    res = run_bass_kernel_spmd(nc, in_maps, list(range(NCORES)), trace=trace)
    out = np.stack([res.results[i]["out"] for i in range(NCORES)], axis=0)
    return out, res


def kernel(**inputs):
    out, _ = kernel_ext(inputs)
    return out
